# revision 5
# baseline (speedup 1.0000x reference)
"""BiAttention (BiDAF-style) layer for Trainium2, data-parallel over batch.

Shapes (hardcoded, from the problem spec):
  encoded_passage  [B=8, P=2048, D=768] f32
  encoded_question [B=8, Q=256,  D=768] f32
  passage_mask     [B=8, P=2048] f32 (binary)
  question_mask    [B=8, Q=256]  f32 (binary)
  output           [B=8, P=2048, 4*D=3072] f32

Each of the 8 NeuronCores processes one batch element; no communication.

The kernel is DMA-roofline bound: 7.1 MB in + 25.2 MB out per core. The
whole program is a single software-pipelined loop over the 16 passage
tiles so that output stores (cols 0:768 passage copy and 768:2304
attention outputs) overlap the attention compute from the first tile on;
only the 6.3 MB of qp-dependent stores (cols 2304:3072) drain at the
end, hidden behind the accumulated DMA backlog.

float32r matmuls run at full PE rate for N>=256 at tf32-like multiply
precision. Walrus requires every producer of an fp32r-matmul operand to
emit fp32r, so operand tiles are allocated fp32r and exact-fp32
consumers read them through a bitcast view.
"""

import numpy as np

B, P, Q, D = 8, 2048, 256, 768
N_CORES = 8
EPS = 1e-07
NEG_VAL = -10000000.0

NT = P // 128  # 16 passage tiles
DC = D // 128  # 6 contraction chunks
QC = Q // 128  # 2 question chunks


def build_nc(repeat=1):
    import concourse.bass as bass
    import concourse.mybir as mybir
    import concourse.tile as tile
    import concourse.bass_isa as bass_isa
    from concourse import bacc
    from concourse.bass import ts
    from concourse.masks import make_identity

    f32 = mybir.dt.float32
    f32r = mybir.dt.float32r
    Alu = mybir.AluOpType
    Act = mybir.ActivationFunctionType
    Axis = mybir.AxisListType

    nc = bacc.Bacc(
        "TRN2",
        target_bir_lowering=False,
        debug=False,
        enable_asserts=False,
        num_devices=N_CORES,
    )

    ep = nc.dram_tensor("encoded_passage", [P, D], f32, kind="ExternalInput").ap()
    eq = nc.dram_tensor("encoded_question", [Q, D], f32, kind="ExternalInput").ap()
    pmsk = nc.dram_tensor("passage_mask", [P], f32, kind="ExternalInput").ap()
    qmsk = nc.dram_tensor("question_mask", [Q], f32, kind="ExternalInput").ap()
    out = nc.dram_tensor("out", [P, 4 * D], f32, kind="ExternalOutput").ap()

    with tile.TileContext(nc) as tc:
        with (
            tc.tile_pool(name="const", bufs=1) as const,
            tc.tile_pool(name="work", bufs=3) as work,
            tc.tile_pool(name="sm", bufs=3) as sm,
            tc.tile_pool(name="small", bufs=4) as small,
            tc.tile_pool(name="store", bufs=3) as store,
            tc.tile_pool(name="psTR", bufs=3, space="PSUM") as psTR,
            tc.tile_pool(name="psSIM", bufs=2, space="PSUM") as psSIM,
            tc.tile_pool(name="psPQ", bufs=1, space="PSUM") as psPQ,
        ):
            # ---- constants / persistent tiles ----
            ident = const.tile([128, 128], f32)
            make_identity(nc, ident)

            pas_all = const.tile([128, NT, D], f32r)  # passage, natural layout
            qnat = const.tile([128, QC, D], f32r)  # question, natural layout
            qT = const.tile([128, DC, Q], f32r)  # qmask * question^T  [d, q]
            qmask_b = const.tile([128, Q], f32)  # question mask bcast over rows
            negm1_all = const.tile([128, NT], f32)  # -max(masked_sim) per tile
            ssum_all = const.tile([128, NT], f32)  # exp-sum per tile
            r_all = const.tile([128, NT], f32)  # 1/(softmax sum + eps)
            qp_bc = const.tile([128, D], f32)  # qp_vector broadcast
            ones_l = const.tile([1, 128], f32r)  # K=1 lhsT for the mask row
            ones_c = const.tile([128, 1], f32)  # partition-sum matmul rhs
            negq_row = const.tile([1, Q], f32r)  # NEG_VAL * (1 - qmask)
            pm_t = const.tile([128, NT], f32)  # passage mask, [p_in_tile, t]
            neg2 = const.tile([128, NT], f32)  # NEG_VAL * (1 - pm)

            pas_f32 = pas_all.bitcast(f32)
            qnat_f32 = qnat.bitcast(f32)

            # ---- prologue: question + mask prep; first passage loads ----
            nc.sync.dma_start(
                out=qnat[:, :, :],
                in_=eq.rearrange("(qc p) c -> p qc c", p=128).bitcast(f32r),
            )
            nc.sync.dma_start(out=qmask_b[:, :], in_=qmsk.partition_broadcast(128))
            pm_nat = const.tile([16, 128], f32)
            nc.sync.dma_start(
                out=pm_nat[:, :], in_=pmsk.rearrange("(t p) -> t p", p=128)
            )
            # first two passage pair loads (the rest are paced by the loop)
            for t0 in (0, 2):
                nc.sync.dma_start(
                    out=pas_all[:, t0 : t0 + 2, :],
                    in_=ep[t0 * 128 : (t0 + 2) * 128, :]
                    .rearrange("(tt p) c -> p tt c", p=128)
                    .bitcast(f32r),
                )

            ones_f = const.tile([1, 128], f32)
            nc.vector.memset(ones_f[:, :], 1.0)
            nc.vector.memset(ones_c[:, :], 1.0)
            nc.vector.tensor_copy(ones_l[:, :], ones_f[:, :])
            negq_f = small.tile([1, Q], f32, tag="negq")
            nc.vector.tensor_scalar(
                out=negq_f[:, :],
                in0=qmask_b[0:1, :],
                scalar1=-NEG_VAL,
                scalar2=NEG_VAL,
                op0=Alu.mult,
                op1=Alu.add,
            )
            nc.vector.tensor_copy(negq_row[:, :], negq_f[:, :])

            # masked question transpose: qT[:, dc, :] = qmask * qnat[:, :, dc].T
            for dc in range(DC):
                ps_q = psTR.tile([128, 3, 128], f32, tag="tr", name=f"ps_q{dc}")
                for qc in range(QC):
                    nc.tensor.transpose(
                        ps_q[:, qc, :], qnat_f32[:, qc, ts(dc, 128)], ident[:, :]
                    )
                nc.vector.tensor_mul(
                    qT[:, dc, :],
                    ps_q.rearrange("p a b -> p (a b)")[:, 0:Q],
                    qmask_b[:, :],
                )

            # passage mask transposed to [p_in_tile, tile] + phase-2 bias
            ps_pm = psSIM.tile([128, Q], f32, tag="sim")
            nc.tensor.transpose(ps_pm[:, 0:16], pm_nat[:, :], ident[0:16, 0:16])
            nc.vector.tensor_copy(pm_t[:, :], ps_pm[:, 0:16])
            nc.vector.tensor_scalar(
                out=neg2[:, :],
                in0=pm_t[:, :],
                scalar1=-NEG_VAL,
                scalar2=NEG_VAL,
                op0=Alu.mult,
                op1=Alu.add,
            )

            tprimes = [None] * NT
            o23ps = [None] * NT

            def front(t):
                """Loads + attention front half for tile t."""
                if t % 2 == 0:
                    if t + 4 < NT:
                        tl = t + 4
                        nc.sync.dma_start(
                            out=pas_all[:, tl : tl + 2, :],
                            in_=ep[tl * 128 : (tl + 2) * 128, :]
                            .rearrange("(tt p) c -> p tt c", p=128)
                            .bitcast(f32r),
                        )
                    # passage copy store (out cols 0:D) for this pair
                    dst1 = out[t * 128 : (t + 2) * 128, 0:D].rearrange(
                        "(tt p) c -> p tt c", p=128
                    )
                    nc.sync.dma_start(out=dst1, in_=pas_f32[:, t : t + 2, :])

                # transpose passage tile: pT[:, dc, :] = pas[:, dc-chunk].T
                pT = work.tile([128, DC, 128], f32r, tag="pT")
                for h in range(2):
                    ps3 = psTR.tile([128, 3, 128], f32, tag="tr", name=f"ps3_{t}_{h}")
                    for j in range(3):
                        dc = 3 * h + j
                        nc.tensor.transpose(
                            ps3[:, j, :], pas_f32[:, t, ts(dc, 128)], ident[:, :]
                        )
                    nc.vector.tensor_copy(
                        pT[:, 3 * h : 3 * h + 3, :], ps3[:, 0:3, :]
                    )

                # masked_sim tile [128, Q] in PSUM:
                #   qmask*(passage @ question^T) + NEG_VAL*(1-qmask)
                ps_sim = psSIM.tile([128, Q], f32, tag="sim")
                for dc in range(DC):
                    nc.tensor.matmul(
                        ps_sim[:, :],
                        lhsT=pT[:, dc, :],
                        rhs=qT[:, dc, :],
                        start=(dc == 0),
                        stop=False,
                    )
                nc.tensor.matmul(
                    ps_sim[:, :],
                    lhsT=ones_l[:, :],
                    rhs=negq_row[:, :],
                    start=False,
                    stop=True,
                )

                # max(masked_sim) is both the softmax shift and qp_similarity
                nc.vector.tensor_reduce(
                    out=negm1_all[:, t : t + 1],
                    in_=ps_sim[:, :],
                    axis=Axis.X,
                    op=Alu.max,
                    negate=True,
                )
                # t' = exp(masked_sim - m1)  (masked entries underflow to 0)
                tprime = sm.tile([128, Q], f32, tag="tp", name=f"tp_{t}")
                nc.scalar.activation(
                    out=tprime[:, :],
                    in_=ps_sim[:, :],
                    func=Act.Exp,
                    bias=negm1_all[:, t : t + 1],
                    scale=1.0,
                    accum_out=ssum_all[:, t : t + 1],
                )
                tprimes[t] = tprime
                se1 = small.tile([128, 1], f32, tag="se")
                nc.vector.tensor_scalar_add(se1[:, :], ssum_all[:, t : t + 1], EPS)
                nc.vector.reciprocal(r_all[:, t : t + 1], se1[:, :])

            def back(t):
                """pq matmul + normalize + products + store for tile t."""
                # transpose t' -> [q, p] for the pq matmul
                psA = psTR.tile([128, 3, 128], f32, tag="tr", name=f"psA_{t}")
                for qc in range(QC):
                    nc.tensor.transpose(
                        psA[:, qc, :], tprimes[t][:, ts(qc, 128)], ident[:, :]
                    )
                aT = work.tile([128, QC, 128], f32r, tag="aT")
                nc.vector.tensor_copy(aT[:, :, :], psA[:, 0:QC, :])

                ps_pqa = psPQ.tile([128, 512], f32, tag="pqa")
                ps_pqb = psPQ.tile([128, 256], f32, tag="pqb")
                for qc in range(QC):
                    st = qc == 0
                    sp = qc == QC - 1
                    nc.tensor.matmul(
                        ps_pqa[:, :],
                        lhsT=aT[:, qc, :],
                        rhs=qnat[:, qc, 0:512],
                        start=st,
                        stop=sp,
                    )
                    nc.tensor.matmul(
                        ps_pqb[:, :],
                        lhsT=aT[:, qc, :],
                        rhs=qnat[:, qc, 512:D],
                        start=st,
                        stop=sp,
                    )

                if t % 2 == 0:
                    o23ps[t] = store.tile(
                        [128, 2, 2 * D], f32, tag="o23", bufs=3, name=f"o23_{t}"
                    )
                o23p = o23ps[t - t % 2]
                o23 = o23p[:, t % 2, :]
                # evict + normalize pq into out cols 768:1536 (scalar engine)
                nc.scalar.mul(o23[:, 0:512], ps_pqa[:, :], r_all[:, t : t + 1])
                nc.scalar.mul(o23[:, 512:D], ps_pqb[:, :], r_all[:, t : t + 1])
                if t % 2 == 1:
                    # passage * pq -> cols 1536:2304, then store the pair
                    nc.vector.tensor_mul(
                        o23p[:, :, D : 2 * D],
                        pas_f32[:, t - 1 : t + 1, :],
                        o23p[:, :, 0:D],
                    )
                    dst23 = out[(t - 1) * 128 : (t + 1) * 128, D : 3 * D].rearrange(
                        "(tt p) c -> p tt c", p=128
                    )
                    nc.sync.dma_start(out=dst23, in_=o23p[:, :, :])

            # ---- fused, software-pipelined main loop ----
            front(0)
            for t in range(1, NT):
                front(t)
                back(t - 1)
            back(NT - 1)

            # ---- phase 2: masked softmax over all P, then qp_vector ----
            im2 = sm.tile([128, NT], f32, tag="im2")
            nc.vector.tensor_scalar_mul(im2[:, :], negm1_all[:, :], -1.0)
            im2b = sm.tile([128, NT], f32, tag="im2b")
            nc.vector.tensor_mul(im2b[:, :], im2[:, :], pm_t[:, :])
            rowmax = small.tile([128, 1], f32, tag="p2")
            nc.vector.tensor_reduce(
                out=rowmax[:, :], in_=im2b[:, :], axis=Axis.X, op=Alu.max
            )
            gmax = small.tile([128, 1], f32, tag="p2")
            nc.gpsimd.partition_all_reduce(
                gmax[:, :], rowmax[:, :], channels=128, reduce_op=bass_isa.ReduceOp.max
            )
            neggmax = small.tile([128, 1], f32, tag="p2")
            nc.vector.tensor_scalar_mul(neggmax[:, :], gmax[:, :], -1.0)
            ms2 = sm.tile([128, NT], f32, tag="ms2")
            nc.vector.tensor_add(ms2[:, :], im2b[:, :], neg2[:, :])
            t2 = sm.tile([128, NT], f32, tag="t2")
            s2row = small.tile([128, 1], f32, tag="p2")
            nc.scalar.activation(
                out=t2[:, :],
                in_=ms2[:, :],
                func=Act.Exp,
                bias=neggmax[:, :],
                scale=1.0,
                accum_out=s2row[:, :],
            )
            t2r = sm.tile([128, NT], f32r, tag="t2r")
            nc.vector.tensor_copy(t2r[:, :], t2[:, :])
            # partition-sum of s2row via a K=128 matmul (cheaper than gpsimd)
            ps_sum = psSIM.tile([128, Q], f32, tag="sim")
            nc.tensor.matmul(
                ps_sum[0:1, 0:1],
                lhsT=s2row[:, :],
                rhs=ones_c[:, :],
                start=True,
                stop=True,
            )
            # unnormalized qp_vector = sum_t t2[:, t]^T @ passage_t
            ps_qp1 = psPQ.tile([128, 512], f32, tag="pqa")
            ps_qp2 = psPQ.tile([128, 256], f32, tag="pqb")
            for t in range(NT):
                st = t == 0
                sp = t == NT - 1
                nc.tensor.matmul(
                    ps_qp1[0:1, :],
                    lhsT=t2r[:, t : t + 1],
                    rhs=pas_all[:, t, 0:512],
                    start=st,
                    stop=sp,
                )
                nc.tensor.matmul(
                    ps_qp2[0:1, :],
                    lhsT=t2r[:, t : t + 1],
                    rhs=pas_all[:, t, 512:D],
                    start=st,
                    stop=sp,
                )
            se2 = small.tile([1, 1], f32, tag="p2s")
            nc.vector.tensor_scalar_add(se2[:, :], ps_sum[0:1, 0:1], EPS)
            r2 = small.tile([1, 1], f32, tag="p2s")
            nc.vector.reciprocal(r2[:, :], se2[:, :])
            qp_sb = sm.tile([1, D], f32, tag="qp_sb")
            nc.vector.tensor_scalar_mul(qp_sb[:, 0:512], ps_qp1[0:1, :], r2[:, :])
            nc.vector.tensor_scalar_mul(qp_sb[:, 512:D], ps_qp2[0:1, :], r2[:, :])
            nc.gpsimd.partition_broadcast(qp_bc[:, :], qp_sb[:, :], channels=128)

            # ---- phase 3: passage * qp_vector products + stores ----
            qp_b2 = bass.AP(
                tensor=qp_bc.tensor,
                offset=qp_bc.offset,
                ap=[[D, 128], [0, 2], [1, D]],
            )
            for g in range(NT // 2):
                t = 2 * g
                big = store.tile([128, 2, D], f32, tag="o4", bufs=3, name=f"o4_{g}")
                eng = nc.gpsimd if g % 4 == 3 else nc.vector
                eng.tensor_mul(big[:, :, :], pas_f32[:, t : t + 2, :], qp_b2)
                dst4 = out[t * 128 : (t + 2) * 128, 3 * D : 4 * D].rearrange(
                    "(tt p) c -> p tt c", p=128
                )
                nc.sync.dma_start(out=dst4, in_=big[:, :, :])

    nc.compile()
    return nc


_NC_CACHE = {}


def _get_nc(repeat=1):
    if repeat not in _NC_CACHE:
        _NC_CACHE[repeat] = build_nc(repeat)
    return _NC_CACHE[repeat]


def kernel(
    encoded_passage: np.ndarray,
    encoded_question: np.ndarray,
    passage_mask: np.ndarray,
    question_mask: np.ndarray,
) -> np.ndarray:
    from concourse.bass_utils import run_bass_kernel_spmd

    nc = _get_nc()
    in_maps = [
        {
            "encoded_passage": np.ascontiguousarray(
                encoded_passage[b], dtype=np.float32
            ),
            "encoded_question": np.ascontiguousarray(
                encoded_question[b], dtype=np.float32
            ),
            "passage_mask": np.ascontiguousarray(passage_mask[b], dtype=np.float32),
            "question_mask": np.ascontiguousarray(question_mask[b], dtype=np.float32),
        }
        for b in range(B)
    ]
    res = run_bass_kernel_spmd(nc, in_maps, core_ids=list(range(N_CORES)))
    return np.stack([res.results[b]["out"] for b in range(B)], axis=0)


# revision 15
# speedup vs baseline: 1.0396x; 1.0396x over previous
"""BiAttention (BiDAF-style) layer for Trainium2, data-parallel over batch.

Shapes (hardcoded, from the problem spec):
  encoded_passage  [B=8, P=2048, D=768] f32
  encoded_question [B=8, Q=256,  D=768] f32
  passage_mask     [B=8, P=2048] f32 (binary)
  question_mask    [B=8, Q=256]  f32 (binary)
  output           [B=8, P=2048, 4*D=3072] f32

Each of the 8 NeuronCores processes one batch element; no communication.

The kernel is DMA-roofline bound: 7.1 MB in + 25.2 MB out per core. The
whole program is a single software-pipelined loop over the 16 passage
tiles so that output stores (cols 0:768 passage copy and 768:2304
attention outputs) overlap the attention compute from the first tile on;
only the 6.3 MB of qp-dependent stores (cols 2304:3072) drain at the
end, hidden behind the accumulated DMA backlog.

float32r matmuls run at full PE rate for N>=256 at tf32-like multiply
precision. Walrus requires every producer of an fp32r-matmul operand to
emit fp32r, so operand tiles are allocated fp32r and exact-fp32
consumers read them through a bitcast view.
"""

import numpy as np

B, P, Q, D = 8, 2048, 256, 768
N_CORES = 8
EPS = 1e-07
NEG_VAL = -10000000.0

NT = P // 128  # 16 passage tiles
DC = D // 128  # 6 contraction chunks
QC = Q // 128  # 2 question chunks


def build_nc(repeat=1):
    import concourse.bass as bass
    import concourse.mybir as mybir
    import concourse.tile as tile
    import concourse.bass_isa as bass_isa
    from concourse import bacc
    from concourse.bass import ts
    from concourse.masks import make_identity

    f32 = mybir.dt.float32
    f32r = mybir.dt.float32r
    Alu = mybir.AluOpType
    Act = mybir.ActivationFunctionType
    Axis = mybir.AxisListType

    nc = bacc.Bacc(
        "TRN2",
        target_bir_lowering=False,
        debug=False,
        enable_asserts=False,
        num_devices=N_CORES,
    )

    ep = nc.dram_tensor("encoded_passage", [P, D], f32, kind="ExternalInput").ap()
    eq = nc.dram_tensor("encoded_question", [Q, D], f32, kind="ExternalInput").ap()
    pmsk = nc.dram_tensor("passage_mask", [P], f32, kind="ExternalInput").ap()
    qmsk = nc.dram_tensor("question_mask", [Q], f32, kind="ExternalInput").ap()
    out = nc.dram_tensor("out", [P, 4 * D], f32, kind="ExternalOutput").ap()

    with tile.TileContext(nc) as tc:
        with (
            tc.tile_pool(name="const", bufs=1) as const,
            tc.tile_pool(name="work", bufs=3) as work,
            tc.tile_pool(name="sm", bufs=3) as sm,
            tc.tile_pool(name="small", bufs=4) as small,
            tc.tile_pool(name="store", bufs=3) as store,
            tc.tile_pool(name="psTR", bufs=3, space="PSUM") as psTR,
            tc.tile_pool(name="psSIM", bufs=2, space="PSUM") as psSIM,
            tc.tile_pool(name="psPQ", bufs=1, space="PSUM") as psPQ,
        ):
            # ---- constants / persistent tiles ----
            # f32r identity: f32r transposes run 1.5 cyc/row vs 2.0 for f32,
            # bit-identical results (f32r is an f32 matmul-mode tag).
            ident_f = const.tile([128, 128], f32)
            make_identity(nc, ident_f)
            ident_r = const.tile([128, 128], f32r)
            nc.vector.tensor_copy(ident_r[:, :], ident_f[:, :])
            ident16 = const.tile([16, 16], f32)
            make_identity(nc, ident16)

            pas_all = const.tile([128, NT, D], f32r)  # passage, natural layout
            qnat = const.tile([128, QC, D], f32r)  # question, natural layout
            qT = const.tile([128, DC, Q], f32r)  # qmask * question^T  [d, q]
            qmask_b = const.tile([128, Q], f32)  # question mask bcast over rows
            negm1_all = const.tile([128, NT], f32)  # -max(masked_sim) per tile
            ssum_all = const.tile([128, NT], f32)  # exp-sum per tile
            r_all = const.tile([128, NT], f32)  # 1/(softmax sum + eps)
            qp_bc = const.tile([128, D], f32)  # qp_vector broadcast
            ones_l = const.tile([1, 128], f32r)  # K=1 lhsT for the mask row
            ones_c = const.tile([128, 1], f32)  # partition-sum matmul rhs
            negq_row = const.tile([1, Q], f32r)  # NEG_VAL * (1 - qmask)
            pm_t = const.tile([128, NT], f32)  # passage mask, [p_in_tile, t]
            neg2 = const.tile([128, NT], f32)  # NEG_VAL * (1 - pm)

            pas_f32 = pas_all.bitcast(f32)

            # ---- prologue: question + mask prep; first passage loads ----
            nc.sync.dma_start(
                out=qnat[:, :, :],
                in_=eq.rearrange("(qc p) c -> p qc c", p=128).bitcast(f32r),
            )
            nc.sync.dma_start(out=qmask_b[:, :], in_=qmsk.partition_broadcast(128))
            pm_nat = const.tile([16, 128], f32)
            nc.sync.dma_start(
                out=pm_nat[:, :], in_=pmsk.rearrange("(t p) -> t p", p=128)
            )
            # first two passage pair loads (the rest are paced by the loop)
            for t0 in (0, 2):
                nc.sync.dma_start(
                    out=pas_all[:, t0 : t0 + 2, :],
                    in_=ep[t0 * 128 : (t0 + 2) * 128, :]
                    .rearrange("(tt p) c -> p tt c", p=128)
                    .bitcast(f32r),
                )

            ones_f = const.tile([1, 128], f32)
            nc.vector.memset(ones_f[:, :], 1.0)
            nc.vector.memset(ones_c[:, :], 1.0)
            nc.vector.tensor_copy(ones_l[:, :], ones_f[:, :])
            negq_f = small.tile([1, Q], f32, tag="negq")
            nc.vector.tensor_scalar(
                out=negq_f[:, :],
                in0=qmask_b[0:1, :],
                scalar1=-NEG_VAL,
                scalar2=NEG_VAL,
                op0=Alu.mult,
                op1=Alu.add,
            )
            nc.vector.tensor_copy(negq_row[:, :], negq_f[:, :])

            # masked question transpose: qT[:, dc, :] = qmask * qnat[:, :, dc].T
            for dc in range(DC):
                ps_q = psTR.tile([128, 3, 128], f32r, tag="tr", name=f"ps_q{dc}")
                for qc in range(QC):
                    nc.tensor.transpose(
                        ps_q[:, qc, :], qnat[:, qc, ts(dc, 128)], ident_r[:, :]
                    )
                nc.vector.tensor_mul(
                    qT[:, dc, :],
                    ps_q.bitcast(f32).rearrange("p a b -> p (a b)")[:, 0:Q],
                    qmask_b[:, :],
                )

            # passage mask transposed to [p_in_tile, tile] + phase-2 bias
            ps_pm = psSIM.tile([128, Q], f32, tag="sim")
            nc.tensor.transpose(ps_pm[:, 0:16], pm_nat[:, :], ident16[:, :])
            nc.vector.tensor_copy(pm_t[:, :], ps_pm[:, 0:16])
            nc.vector.tensor_scalar(
                out=neg2[:, :],
                in0=pm_t[:, :],
                scalar1=-NEG_VAL,
                scalar2=NEG_VAL,
                op0=Alu.mult,
                op1=Alu.add,
            )

            tprimes = [None] * NT
            o23ps = [None] * NT

            def front(t):
                """Loads + attention front half for tile t."""
                if t % 2 == 0:
                    if t + 4 < NT:
                        tl = t + 4
                        nc.sync.dma_start(
                            out=pas_all[:, tl : tl + 2, :],
                            in_=ep[tl * 128 : (tl + 2) * 128, :]
                            .rearrange("(tt p) c -> p tt c", p=128)
                            .bitcast(f32r),
                        )
                    # passage copy store (out cols 0:D) for this pair
                    dst1 = out[t * 128 : (t + 2) * 128, 0:D].rearrange(
                        "(tt p) c -> p tt c", p=128
                    )
                    nc.sync.dma_start(out=dst1, in_=pas_f32[:, t : t + 2, :])

                # transpose passage tile: pT[:, dc, :] = pas[:, dc-chunk].T
                pT = work.tile([128, DC, 128], f32r, tag="pT")
                for h in range(2):
                    ps3 = psTR.tile([128, 3, 128], f32r, tag="tr", name=f"ps3_{t}_{h}")
                    for j in range(3):
                        dc = 3 * h + j
                        nc.tensor.transpose(
                            ps3[:, j, :], pas_all[:, t, ts(dc, 128)], ident_r[:, :]
                        )
                    nc.vector.tensor_copy(
                        pT[:, 3 * h : 3 * h + 3, :], ps3.bitcast(f32)[:, 0:3, :]
                    )

                # masked_sim tile [128, Q] in PSUM:
                #   qmask*(passage @ question^T) + NEG_VAL*(1-qmask)
                ps_sim = psSIM.tile([128, Q], f32, tag="sim")
                for dc in range(DC):
                    nc.tensor.matmul(
                        ps_sim[:, :],
                        lhsT=pT[:, dc, :],
                        rhs=qT[:, dc, :],
                        start=(dc == 0),
                        stop=False,
                    )
                nc.tensor.matmul(
                    ps_sim[:, :],
                    lhsT=ones_l[:, :],
                    rhs=negq_row[:, :],
                    start=False,
                    stop=True,
                )

                # max(masked_sim) is both the softmax shift and qp_similarity
                nc.vector.tensor_reduce(
                    out=negm1_all[:, t : t + 1],
                    in_=ps_sim[:, :],
                    axis=Axis.X,
                    op=Alu.max,
                    negate=True,
                )
                # t' = exp(masked_sim - m1)  (masked entries underflow to 0)
                tprime = sm.tile([128, Q], f32r, tag="tp", name=f"tp_{t}")
                nc.scalar.activation(
                    out=tprime[:, :],
                    in_=ps_sim[:, :],
                    func=Act.Exp,
                    bias=negm1_all[:, t : t + 1],
                    scale=1.0,
                    accum_out=ssum_all[:, t : t + 1],
                )
                tprimes[t] = tprime
                se1 = small.tile([128, 1], f32, tag="se")
                nc.vector.tensor_scalar_add(se1[:, :], ssum_all[:, t : t + 1], EPS)
                nc.vector.reciprocal(r_all[:, t : t + 1], se1[:, :])

            def back(t):
                """pq matmul + normalize + products + store for tile t."""
                # transpose t' -> [q, p] for the pq matmul
                psA = psTR.tile([128, 3, 128], f32r, tag="tr", name=f"psA_{t}")
                for qc in range(QC):
                    nc.tensor.transpose(
                        psA[:, qc, :], tprimes[t][:, ts(qc, 128)], ident_r[:, :]
                    )
                aT = work.tile([128, QC, 128], f32r, tag="aT")
                nc.vector.tensor_copy(aT[:, :, :], psA.bitcast(f32)[:, 0:QC, :])

                ps_pqa = psPQ.tile([128, 512], f32, tag="pqa")
                ps_pqb = psPQ.tile([128, 256], f32, tag="pqb")
                for qc in range(QC):
                    st = qc == 0
                    sp = qc == QC - 1
                    nc.tensor.matmul(
                        ps_pqa[:, :],
                        lhsT=aT[:, qc, :],
                        rhs=qnat[:, qc, 0:512],
                        start=st,
                        stop=sp,
                    )
                    nc.tensor.matmul(
                        ps_pqb[:, :],
                        lhsT=aT[:, qc, :],
                        rhs=qnat[:, qc, 512:D],
                        start=st,
                        stop=sp,
                    )

                if t % 2 == 0:
                    o23ps[t] = store.tile(
                        [128, 2, 2 * D], f32, tag="o23", bufs=3, name=f"o23_{t}"
                    )
                o23p = o23ps[t - t % 2]
                o23 = o23p[:, t % 2, :]
                # evict + normalize pq into out cols 768:1536 (scalar engine)
                nc.scalar.mul(o23[:, 0:512], ps_pqa[:, :], r_all[:, t : t + 1])
                nc.scalar.mul(o23[:, 512:D], ps_pqb[:, :], r_all[:, t : t + 1])
                if t % 2 == 1:
                    # passage * pq -> cols 1536:2304, then store the pair
                    nc.vector.tensor_mul(
                        o23p[:, :, D : 2 * D],
                        pas_f32[:, t - 1 : t + 1, :],
                        o23p[:, :, 0:D],
                    )
                    dst23 = out[(t - 1) * 128 : (t + 1) * 128, D : 3 * D].rearrange(
                        "(tt p) c -> p tt c", p=128
                    )
                    nc.sync.dma_start(out=dst23, in_=o23p[:, :, :])

            # ---- fused, software-pipelined main loop ----
            front(0)
            for t in range(1, NT):
                front(t)
                back(t - 1)
            back(NT - 1)

            # ---- phase 2: masked softmax over all P, then qp_vector ----
            im2 = sm.tile([128, NT], f32, tag="im2")
            nc.vector.tensor_scalar_mul(im2[:, :], negm1_all[:, :], -1.0)
            im2b = sm.tile([128, NT], f32, tag="im2b")
            nc.vector.tensor_mul(im2b[:, :], im2[:, :], pm_t[:, :])
            rowmax = small.tile([128, 1], f32, tag="p2")
            nc.vector.tensor_reduce(
                out=rowmax[:, :], in_=im2b[:, :], axis=Axis.X, op=Alu.max
            )
            gmax = small.tile([128, 1], f32, tag="p2")
            nc.gpsimd.partition_all_reduce(
                gmax[:, :], rowmax[:, :], channels=128, reduce_op=bass_isa.ReduceOp.max
            )
            neggmax = small.tile([128, 1], f32, tag="p2")
            nc.vector.tensor_scalar_mul(neggmax[:, :], gmax[:, :], -1.0)
            ms2 = sm.tile([128, NT], f32, tag="ms2")
            nc.vector.tensor_add(ms2[:, :], im2b[:, :], neg2[:, :])
            t2 = sm.tile([128, NT], f32, tag="t2")
            s2row = small.tile([128, 1], f32, tag="p2")
            nc.scalar.activation(
                out=t2[:, :],
                in_=ms2[:, :],
                func=Act.Exp,
                bias=neggmax[:, :],
                scale=1.0,
                accum_out=s2row[:, :],
            )
            t2r = sm.tile([128, NT], f32r, tag="t2r")
            nc.vector.tensor_copy(t2r[:, :], t2[:, :])
            # partition-sum of s2row via a K=128 matmul (cheaper than gpsimd)
            ps_sum = psSIM.tile([128, Q], f32, tag="sim")
            nc.tensor.matmul(
                ps_sum[0:1, 0:1],
                lhsT=s2row[:, :],
                rhs=ones_c[:, :],
                start=True,
                stop=True,
            )
            # unnormalized qp_vector = sum_t t2[:, t]^T @ passage_t
            ps_qp1 = psPQ.tile([128, 512], f32, tag="pqa")
            ps_qp2 = psPQ.tile([128, 256], f32, tag="pqb")
            for t in range(NT):
                st = t == 0
                sp = t == NT - 1
                nc.tensor.matmul(
                    ps_qp1[0:1, :],
                    lhsT=t2r[:, t : t + 1],
                    rhs=pas_all[:, t, 0:512],
                    start=st,
                    stop=sp,
                )
                nc.tensor.matmul(
                    ps_qp2[0:1, :],
                    lhsT=t2r[:, t : t + 1],
                    rhs=pas_all[:, t, 512:D],
                    start=st,
                    stop=sp,
                )
            se2 = small.tile([1, 1], f32, tag="p2s")
            nc.vector.tensor_scalar_add(se2[:, :], ps_sum[0:1, 0:1], EPS)
            r2 = small.tile([1, 1], f32, tag="p2s")
            nc.vector.reciprocal(r2[:, :], se2[:, :])
            # normalize + evict on the scalar engine, then broadcast across
            # partitions with a K=1 ones matmul (much faster than gpsimd)
            qp_sb = sm.tile([1, D], f32, tag="qp_sb")
            nc.scalar.mul(qp_sb[:, 0:512], ps_qp1[0:1, :], r2[:, :])
            nc.scalar.mul(qp_sb[:, 512:D], ps_qp2[0:1, :], r2[:, :])
            ps_qb1 = psPQ.tile([128, 512], f32, tag="pqa")
            ps_qb2 = psPQ.tile([128, 256], f32, tag="pqb")
            nc.tensor.matmul(
                ps_qb1[:, :], lhsT=ones_f[:, :], rhs=qp_sb[:, 0:512],
                start=True, stop=True,
            )
            nc.tensor.matmul(
                ps_qb2[:, :], lhsT=ones_f[:, :], rhs=qp_sb[:, 512:D],
                start=True, stop=True,
            )
            nc.vector.tensor_copy(qp_bc[:, 0:512], ps_qb1[:, :])
            nc.vector.tensor_copy(qp_bc[:, 512:D], ps_qb2[:, :])

            # ---- phase 3: passage * qp_vector products + stores ----
            qp_b2 = bass.AP(
                tensor=qp_bc.tensor,
                offset=qp_bc.offset,
                ap=[[D, 128], [0, 2], [1, D]],
            )
            for g in range(NT // 2):
                t = 2 * g
                big = store.tile([128, 2, D], f32, tag="o4", bufs=3, name=f"o4_{g}")
                nc.vector.tensor_mul(big[:, :, :], pas_f32[:, t : t + 2, :], qp_b2)
                dst4 = out[t * 128 : (t + 2) * 128, 3 * D : 4 * D].rearrange(
                    "(tt p) c -> p tt c", p=128
                )
                nc.sync.dma_start(out=dst4, in_=big[:, :, :])

    nc.compile()
    return nc


_NC_CACHE = {}


def _get_nc(repeat=1):
    if repeat not in _NC_CACHE:
        _NC_CACHE[repeat] = build_nc(repeat)
    return _NC_CACHE[repeat]


def kernel(
    encoded_passage: np.ndarray,
    encoded_question: np.ndarray,
    passage_mask: np.ndarray,
    question_mask: np.ndarray,
) -> np.ndarray:
    from concourse.bass_utils import run_bass_kernel_spmd

    nc = _get_nc()
    in_maps = [
        {
            "encoded_passage": np.ascontiguousarray(
                encoded_passage[b], dtype=np.float32
            ),
            "encoded_question": np.ascontiguousarray(
                encoded_question[b], dtype=np.float32
            ),
            "passage_mask": np.ascontiguousarray(passage_mask[b], dtype=np.float32),
            "question_mask": np.ascontiguousarray(question_mask[b], dtype=np.float32),
        }
        for b in range(B)
    ]
    res = run_bass_kernel_spmd(nc, in_maps, core_ids=list(range(N_CORES)))
    return np.stack([res.results[b]["out"] for b in range(B)], axis=0)
